# revision 84
# baseline (speedup 1.0000x reference)
"""Grouped GEMM (MoE expert matmul) on 8 TRN2 NeuronCores.

Problem: a [66048, 1024] f32 tokens, b [8, 1024, 1024] f32 expert weights,
static uneven per-expert token counts. d[m] = a[m] @ b[expert(m)].

Strategy (expert-parallel via M-sharding, zero collectives):
- Token rows are assigned host-side to 8 cores x 3 "slots" of (6, 22, 37)
  m-tiles (128 rows each) = 65 tiles/core. Every slot is single-expert;
  each core receives the 3 expert matrices its slots need. The
  (core,slot)->expert binding is pure DATA, so one SPMD program serves
  all cores. Only 4 of 520 tiles are zero-padding.
- A is pre-transposed host-side into per-tile lhsT layout [ki, ko, mm]
  (so the PE does no transposes at all) and split into fp8-e4m3 hi+lo
  (a ~= a_h + a_l); B likewise. The product is computed as
      d ~= a_h@b_h + a_l@b_h + a_h@b_l
  with all terms as fp8 DoubleRow matmuls (2 k-tiles per instruction,
  0.5 cycles/row) accumulating into the same PSUM bank. The b_l
  correction covers 3 of 4 k-pairs, dropped to 2 in half the tiles
  (BL2_TILES): rel err 1.63e-2 vs the 2e-2 gate, at 20-22 matmul
  instructions per tile (bf16-equivalent cost 32) -> per-core PE floor
  ~146us.
- Per m-tile: 20-22 DoubleRow matmuls (2 psum halves x (4+4+blp)
  k-pair chains), PSUM evicted to SBUF as bf16 by DVE, stored by HWDGE
  DMA on
  the sync queue; d upcast host-side. A streams in 5-tile SWDGE chunks
  with 3-chunk prefetch; the startup DMA train is ordered so each piece
  lands exactly when its first consumers need it (b0h half on the sync
  HWDGE queue first, 4-tile A piece next); warmup matmuls on zeroed
  tiles burn the PE p-state ramp during the startup loads; the last
  tile's nh1 runs as two 256-wide chains so its eviction+stores overlap
  the final matmuls (shorter kernel tail).
"""

import numpy as np

GROUP_SIZES = [12288, 10240, 9216, 8192, 7168, 7168, 6144, 5632]
OFFSETS = np.concatenate([[0], np.cumsum(GROUP_SIZES)]).astype(np.int64)
M_TOTAL = int(OFFSETS[-1])  # 66048
K = 1024
N = 1024
E = 8
P = 128
KK = K // P  # 8 k-tiles
NH = 2  # two 512-wide psum halves

# Per-core uniform slot structure, in m-tiles of 128 rows.
SLOT_TILES = (6, 22, 37)  # sum = 65 tiles = 8320 rows per core
TILES_PER_CORE = sum(SLOT_TILES)
ROWS_PER_CORE = TILES_PER_CORE * P
SLOT_ROW_OFF = (0, SLOT_TILES[0] * P, (SLOT_TILES[0] + SLOT_TILES[1]) * P)

CHUNK = 5  # m-tiles per A-load DMA; 13 chunks cover 65 tiles
NCHUNKS = TILES_PER_CORE // CHUNK
PREFETCH = 3  # chunks issued ahead of consumption
WARMUP = 30  # dummy PE matmuls burning the p-state ramp during startup DMA

# k-pairs (of 4) covered by the a_h@b_l correction term. 4 -> rel err
# ~2.1e-3; 3 -> ~1.34e-2 (vs the 2e-2 gate) and 2 DR matmuls/tile fewer.
BL_PAIRS = 3
# Tiles where the b_l correction drops one more k-pair (coverage 2 of
# 4). Each such tile saves 2 DR matmuls; 32 of 65 tiles -> rel err
# ~1.64e-2, still 1.22x under the gate. The last tile keeps 3 pairs so
# the tuned tail structure is unchanged.
BL2_TILES = frozenset(t for t in range(TILES_PER_CORE - 1) if t % 2 == 0)

# expert id for (slot, core): found by exact-cover search; 4 pad tiles total.
SLOT_EXPERT = (
    (1, 3, 4, 4, 5, 5, 6, 6),  # slot 0: 6 tiles each
    (0, 3, 4, 4, 5, 5, 7, 7),  # slot 1: 22 tiles each
    (0, 0, 1, 1, 2, 2, 3, 6),  # slot 2: 37 tiles each
)


def _build_schedule():
    """Returns list of (core, slot, slot_row_start, global_row_start, nrows)."""
    cursor = [int(OFFSETS[e]) for e in range(E)]
    recs = []
    # Deterministic fill order: slot index, then core.
    for s in range(3):
        for c in range(8):
            e = SLOT_EXPERT[s][c]
            cap = SLOT_TILES[s] * P
            take = min(cap, int(OFFSETS[e + 1]) - cursor[e])
            if take > 0:
                recs.append((c, s, SLOT_ROW_OFF[s], cursor[e], take))
                cursor[e] += take
    for e in range(E):
        assert cursor[e] == int(OFFSETS[e + 1]), (e, cursor[e])
    return recs


_SCHEDULE = _build_schedule()


def _build_bass():
    import concourse.bass as bass  # noqa: F401
    import concourse.mybir as mybir
    import concourse.tile as tile
    from concourse import bacc

    f32 = mybir.dt.float32
    bf16 = mybir.dt.bfloat16
    f8 = mybir.dt.float8e4

    nc = bacc.Bacc(
        "TRN2", target_bir_lowering=False, debug=False, enable_asserts=False
    )

    # A in pre-transposed lhsT layout: row (t*128 + ki) holds the 1024
    # values [ko, mm] of tile t; hi and lo fp8 planes.
    ah = nc.dram_tensor("ah", [ROWS_PER_CORE, K], f8, kind="ExternalInput").ap()
    al = nc.dram_tensor("al", [ROWS_PER_CORE, K], f8, kind="ExternalInput").ap()
    # B per slot: row (ki*8 + ko) holds the 1024 n-values; hi and lo.
    bhs = [
        nc.dram_tensor(f"bh{s}", [P * KK, N], f8, kind="ExternalInput").ap()
        for s in range(3)
    ]
    bls = [
        nc.dram_tensor(f"bl{s}", [P * KK, N], f8, kind="ExternalInput").ap()
        for s in range(3)
    ]
    d = nc.dram_tensor("d", [ROWS_PER_CORE, N], bf16, kind="ExternalOutput").ap()

    # which slot (-> b input) each m-tile uses (static, uniform across cores)
    tile_slot = []
    for s in range(3):
        tile_slot += [s] * SLOT_TILES[s]

    from contextlib import ExitStack

    with tile.TileContext(nc) as tc, ExitStack() as ctx:
        bpool = ctx.enter_context(tc.tile_pool(name="bpool", bufs=1))
        apool = ctx.enter_context(tc.tile_pool(name="apool", bufs=4))
        psd = ctx.enter_context(tc.tile_pool(name="psd", bufs=8, space="PSUM"))
        # Deep store staging: early DMA-engine time is monopolized by the
        # B/A loads, so d-stores queue up; 24 bufs (48KB) of slack keep the
        # eviction copies (and thus PSUM recycling) from backpressuring PE.
        dpool = ctx.enter_context(tc.tile_pool(name="dpool", bufs=24))

        # First load issued before anything else: b0h[ko0-3] on the sync
        # HWDGE queue, so SP's DMA issue chain starts at t=0.
        HB = KK * N // 2
        bt00 = bpool.tile([P, KK, N], f8, name="b0_0")
        b00f = bt00[:].rearrange("ki ko n -> ki (ko n)")
        b00in = bhs[0].rearrange("(ki ko) n -> ki (ko n)", ko=KK)
        nc.sync.dma_start(out=b00f[:, :HB], in_=b00in[:, :HB])

        # Warmup: the PE p-state ramps to full clock only after 3us of
        # continuous execution. Dummy DoubleRow matmuls on zeroed tiles
        # keep the PE busy (and ramping) while the first B/A DMAs land,
        # so the real matmul stream starts at full speed with no idle gap.
        wa = bpool.tile([P, 2, P], f8, name="wa")
        wb = bpool.tile([P, 2, 256], f8, name="wb")
        nc.vector.memset(wa[:], 0.0)
        nc.vector.memset(wb[:], 0.0)
        wp = psd.tile([P, 512], f32, name="ps")
        for _ in range(WARMUP):
            nc.tensor.matmul(
                wp[:, :256],
                wa[:],
                wb[:],
                start=True,
                stop=True,
                perf_mode=mybir.MatmulPerfMode.DoubleRow,
            )

        b_sb = {}  # (slot, lvl) -> [128, KK, N] fp8 tile

        def load_b(s, lvl, pieces=None, queues=None):
            src = (bhs if lvl == 0 else bls)[s]
            bt = bpool.tile([P, KK, N], f8, name=f"b{lvl}_{s}")
            # Flat [128, 8KB] view: per-partition lines are contiguous in
            # both DRAM and SBUF, so the DMA needs 128 descriptors, not
            # 1024 — shorter SWDGE descriptor-generation on the Pool SEQ.
            # Lo planes: ko-tiles >= 2*BL_PAIRS are never read (the b_l
            # correction only covers BL_PAIRS k-pairs), so don't load them.
            out_f = bt[:].rearrange("ki ko n -> ki (ko n)")
            in_f = src.rearrange("(ki ko) n -> ki (ko n)", ko=KK)
            end = (KK if lvl == 0 else 2 * BL_PAIRS) * N
            pieces = pieces or [(0, end)]
            for i, (p0, p1) in enumerate(pieces):
                queue = (queues or {}).get(i, nc.gpsimd)
                queue.dma_start(out=out_f[:, p0:p1], in_=in_f[:, p0:p1])
            b_sb[(s, lvl)] = bt

        a_ch = {}  # (chunk, lvl) -> [128, CHUNK, KK, 128] fp8 tile

        def load_chunk(c, pieces=None, queues=None, lvls=(0, 1)):
            # pieces: tile sub-ranges loaded as separate DMAs so their
            # consumers (tracked per sub-tile range) unblock early.
            for lvl in lvls:
                pool, src = ((apool, ah), (apool, al))[lvl]
                at = a_ch.get((c, lvl))
                if at is None:
                    at = pool.tile([P, CHUNK, KK, P], f8, name=f"a{lvl}")
                    a_ch[(c, lvl)] = at
                for i, (p0, p1) in enumerate(pieces or [(0, CHUNK)]):
                    queue = (queues or {}).get(i, nc.gpsimd)
                    queue.dma_start(
                        out=at[:, p0:p1],
                        in_=src[
                            (c * CHUNK + p0) * P : (c * CHUNK + p1) * P, :
                        ].rearrange("(c ki) (ko mm) -> ki c ko mm", ki=P, ko=KK),
                    )

        # Startup: the serialized DMA train is ordered so that each piece
        # lands just before its first consumer instructions need it, with
        # the two lead pieces on HWDGE queues (shorter issue lead than a
        # SWDGE prep). Desired grant order: b0h[ko0-3] (sync), a0_hi
        # tiles0-1 (scalar), then the SWDGE train: b0h[ko4-7], a0_lo
        # tiles0-1, b0l in halves, a0 tiles2-4, b1, chunks 1-2, b2 later.
        # Grant order on the serialized DMA engine follows request order:
        # b0h[ko0-3] via sync HWDGE (shortest lead), then the SWDGE preps
        # in emission order. The first A piece is prepped BEFORE b0h's
        # second half so the first matmuls (jj0/jj1 of tiles 0-1) can
        # start ~1.4us earlier; the scheduler hoists them over the wait
        # for b0h[ko4-7].
        b_sb[(0, 0)] = bt00
        load_chunk(0, pieces=[(0, 4)], lvls=(0,))
        nc.gpsimd.dma_start(out=b00f[:, HB:], in_=b00in[:, HB:])
        load_chunk(0, pieces=[(0, 4)], lvls=(1,))
        load_b(0, 1)
        load_chunk(0, pieces=[(4, CHUNK)])
        load_b(1, 0)
        load_chunk(1)
        load_b(1, 1)
        load_chunk(2)

        for t in range(TILES_PER_CORE):
            c, j = divmod(t, CHUNK)
            if j == 0 and c + PREFETCH < NCHUNKS:
                load_chunk(c + PREFETCH)
            if t == 8:
                load_b(2, 0)
                load_b(2, 1)
            s = tile_slot[t]
            at_h = a_ch[(c, 0)]
            at_l = a_ch[(c, 1)]
            b_h = b_sb[(s, 0)]
            b_l = b_sb[(s, 1)]
            last = t == TILES_PER_CORE - 1
            ps0 = psd.tile([P, 512], f32, name="ps")
            if not last:
                ps1 = psd.tile([P, 512], f32, name="ps")
                chains = ((ps0[:], 0, 512), (ps1[:], 512, 1024))
            else:
                # Last tile: nh1 as two 256-wide chains (same PE cycles) so
                # the first half's eviction+store pipeline overlaps the
                # second half's matmuls, shortening the kernel tail. psq0
                # reuses the warmup bank (free after warmup).
                psq0 = psd.tile([P, 512], f32, name="ps")
                psq1 = psd.tile([P, 512], f32, name="ps")
                chains = (
                    (ps0[:], 0, 512),
                    (psq0[:, :256], 512, 768),
                    (psq1[:, :256], 768, 1024),
                )
            blp = 2 if t in BL2_TILES else BL_PAIRS
            nchain = 2 * (KK // 2) + blp
            for pst, n0, n1 in chains:
                idx = 0
                for w_t, r_t, npairs in (
                    (at_h, b_h, KK // 2),
                    (at_l, b_h, KK // 2),
                    (at_h, b_l, blp),
                ):
                    for jj in range(npairs):
                        nc.tensor.matmul(
                            pst,
                            w_t[:, j, 2 * jj : 2 * jj + 2, :],
                            r_t[:, 2 * jj : 2 * jj + 2, n0:n1],
                            start=(idx == 0),
                            stop=(idx == nchain - 1),
                            perf_mode=mybir.MatmulPerfMode.DoubleRow,
                        )
                        idx += 1
            d_sb = dpool.tile([P, N], bf16, name="d_sb")
            if not last:
                nc.vector.tensor_copy(d_sb[:, :512], ps0[:])
                nc.vector.tensor_copy(d_sb[:, 512:], ps1[:])
                nc.sync.dma_start(out=d[t * P : (t + 1) * P, :], in_=d_sb[:])
            else:
                # Each piece is evicted and stored as soon as its chain
                # stops; only the final 256-wide piece trails the last
                # matmul.
                nc.vector.tensor_copy(d_sb[:, :512], ps0[:])
                nc.sync.dma_start(
                    out=d[t * P : (t + 1) * P, :512], in_=d_sb[:, :512]
                )
                nc.vector.tensor_copy(d_sb[:, 512:768], psq0[:, :256])
                nc.vector.tensor_copy(d_sb[:, 768:], psq1[:, :256])
                nc.sync.dma_start(
                    out=d[t * P : (t + 1) * P, 512:], in_=d_sb[:, 512:]
                )
            # free the chunk dict entries we no longer need
            if j == CHUNK - 1:
                a_ch.pop((c, 0), None)
                a_ch.pop((c, 1), None)

    nc.compile()
    return nc


_NC_CACHE = None


def _prep_inputs(a, b):
    """Host-side shard + transpose + fp8 hi/lo split. Returns in_maps."""
    import ml_dtypes

    f8 = ml_dtypes.float8_e4m3

    a32 = np.ascontiguousarray(np.asarray(a), dtype=np.float32)
    b32 = np.ascontiguousarray(np.asarray(b), dtype=np.float32)
    assert a32.shape == (M_TOTAL, K), a32.shape
    assert b32.shape == (E, K, N), b32.shape

    a_h = a32.astype(f8)
    a_l = (a32 - a_h.astype(np.float32)).astype(f8)
    b_h = b32.astype(f8)
    b_l = (b32 - b_h.astype(np.float32)).astype(f8)

    # Per-expert B in [ki, ko, n] lhs-contraction layout, flattened 2D.
    def prep_b(x):  # x: [K, N] fp8
        return np.ascontiguousarray(
            x.reshape(KK, P, N).transpose(1, 0, 2).reshape(P * KK, N)
        )

    b_h_prep = [prep_b(b_h[e]) for e in range(E)]
    b_l_prep = [prep_b(b_l[e]) for e in range(E)]

    # Per-core A shards (zero-padded), then per-tile transpose to
    # [t, ki, ko, mm] flattened to [(t ki), (ko mm)].
    def prep_a(x):  # x: [ROWS_PER_CORE, K] fp8
        y = x.reshape(TILES_PER_CORE, P, KK, P).transpose(0, 3, 2, 1)
        return np.ascontiguousarray(y).reshape(ROWS_PER_CORE, K)

    in_maps = []
    for c in range(8):
        sh_h = np.zeros((ROWS_PER_CORE, K), dtype=f8)
        sh_l = np.zeros((ROWS_PER_CORE, K), dtype=f8)
        for cc, s, soff, goff, n in _SCHEDULE:
            if cc == c:
                sh_h[soff : soff + n] = a_h[goff : goff + n]
                sh_l[soff : soff + n] = a_l[goff : goff + n]
        m = {"ah": prep_a(sh_h), "al": prep_a(sh_l)}
        for s in range(3):
            e = SLOT_EXPERT[s][c]
            m[f"bh{s}"] = b_h_prep[e]
            m[f"bl{s}"] = b_l_prep[e]
        in_maps.append(m)
    return in_maps


def kernel(a, b):
    global _NC_CACHE
    from concourse.bass_utils import run_bass_kernel_spmd

    if _NC_CACHE is None:
        _NC_CACHE = _build_bass()
    nc = _NC_CACHE

    in_maps = _prep_inputs(a, b)
    res = run_bass_kernel_spmd(nc, in_maps, core_ids=list(range(8)))

    out = np.empty((M_TOTAL, N), dtype=np.float32)
    for c, s, soff, goff, n in _SCHEDULE:
        out[goff : goff + n] = res.results[c]["d"][soff : soff + n].astype(
            np.float32
        )
    return out


# revision 85
# speedup vs baseline: 1.0111x; 1.0111x over previous
"""Grouped GEMM (MoE expert matmul) on 8 TRN2 NeuronCores.

Problem: a [66048, 1024] f32 tokens, b [8, 1024, 1024] f32 expert weights,
static uneven per-expert token counts. d[m] = a[m] @ b[expert(m)].

Strategy (expert-parallel via M-sharding, zero collectives):
- Token rows are assigned host-side to 8 cores x 3 "slots" of (6, 22, 37)
  m-tiles (128 rows each) = 65 tiles/core. Every slot is single-expert;
  each core receives the 3 expert matrices its slots need. The
  (core,slot)->expert binding is pure DATA, so one SPMD program serves
  all cores. Only 4 of 520 tiles are zero-padding.
- A is pre-transposed host-side into per-tile lhsT layout [ki, ko, mm]
  (so the PE does no transposes at all) and split into fp8-e4m3 hi+lo
  (a ~= a_h + a_l); B likewise. The product is computed as
      d ~= a_h@b_h + a_l@b_h + a_h@b_l
  with all terms as fp8 DoubleRow matmuls (2 k-tiles per instruction,
  0.5 cycles/row) accumulating into the same PSUM bank. The b_l
  correction covers 3 of 4 k-pairs, dropped to 2 in half the tiles
  (BL2_TILES): rel err 1.63e-2 vs the 2e-2 gate, at 20-22 matmul
  instructions per tile (bf16-equivalent cost 32) -> per-core PE floor
  ~146us.
- Per m-tile: 20-22 DoubleRow matmuls (2 psum halves x (4+4+blp)
  k-pair chains), PSUM evicted to SBUF as bf16 by DVE, stored by HWDGE
  DMA on
  the sync queue; d upcast host-side. A streams in 5-tile SWDGE chunks
  with 3-chunk prefetch; the startup DMA train is ordered so each piece
  lands exactly when its first consumers need it (b0h half on the sync
  HWDGE queue first, 4-tile A piece next); warmup matmuls on zeroed
  tiles burn the PE p-state ramp during the startup loads; the last
  tile's nh1 runs as two 256-wide chains so its eviction+stores overlap
  the final matmuls (shorter kernel tail).
"""

import numpy as np

GROUP_SIZES = [12288, 10240, 9216, 8192, 7168, 7168, 6144, 5632]
OFFSETS = np.concatenate([[0], np.cumsum(GROUP_SIZES)]).astype(np.int64)
M_TOTAL = int(OFFSETS[-1])  # 66048
K = 1024
N = 1024
E = 8
P = 128
KK = K // P  # 8 k-tiles
NH = 2  # two 512-wide psum halves

# Per-core uniform slot structure, in m-tiles of 128 rows.
SLOT_TILES = (6, 22, 37)  # sum = 65 tiles = 8320 rows per core
TILES_PER_CORE = sum(SLOT_TILES)
ROWS_PER_CORE = TILES_PER_CORE * P
SLOT_ROW_OFF = (0, SLOT_TILES[0] * P, (SLOT_TILES[0] + SLOT_TILES[1]) * P)

CHUNK = 5  # m-tiles per A-load DMA; 13 chunks cover 65 tiles
NCHUNKS = TILES_PER_CORE // CHUNK
PREFETCH = 3  # chunks issued ahead of consumption
WARMUP = 30  # dummy PE matmuls burning the p-state ramp during startup DMA

# k-pairs (of 4) covered by the a_h@b_l correction term. 4 -> rel err
# ~2.1e-3; 3 -> ~1.34e-2 (vs the 2e-2 gate) and 2 DR matmuls/tile fewer.
BL_PAIRS = 3
# Tiles where the b_l correction drops one more k-pair (coverage 2 of
# 4). Each such tile saves 2 DR matmuls; 40 of 65 tiles -> rel err
# ~1.70e-2, still 1.17x under the gate. The last tile keeps 3 pairs so
# the tuned tail structure is unchanged.
BL2_TILES = frozenset(
    t for t in range(TILES_PER_CORE - 1) if t % 2 == 0 or t < 16
)

# expert id for (slot, core): found by exact-cover search; 4 pad tiles total.
SLOT_EXPERT = (
    (1, 3, 4, 4, 5, 5, 6, 6),  # slot 0: 6 tiles each
    (0, 3, 4, 4, 5, 5, 7, 7),  # slot 1: 22 tiles each
    (0, 0, 1, 1, 2, 2, 3, 6),  # slot 2: 37 tiles each
)


def _build_schedule():
    """Returns list of (core, slot, slot_row_start, global_row_start, nrows)."""
    cursor = [int(OFFSETS[e]) for e in range(E)]
    recs = []
    # Deterministic fill order: slot index, then core.
    for s in range(3):
        for c in range(8):
            e = SLOT_EXPERT[s][c]
            cap = SLOT_TILES[s] * P
            take = min(cap, int(OFFSETS[e + 1]) - cursor[e])
            if take > 0:
                recs.append((c, s, SLOT_ROW_OFF[s], cursor[e], take))
                cursor[e] += take
    for e in range(E):
        assert cursor[e] == int(OFFSETS[e + 1]), (e, cursor[e])
    return recs


_SCHEDULE = _build_schedule()


def _build_bass():
    import concourse.bass as bass  # noqa: F401
    import concourse.mybir as mybir
    import concourse.tile as tile
    from concourse import bacc

    f32 = mybir.dt.float32
    bf16 = mybir.dt.bfloat16
    f8 = mybir.dt.float8e4

    nc = bacc.Bacc(
        "TRN2", target_bir_lowering=False, debug=False, enable_asserts=False
    )

    # A in pre-transposed lhsT layout: row (t*128 + ki) holds the 1024
    # values [ko, mm] of tile t; hi and lo fp8 planes.
    ah = nc.dram_tensor("ah", [ROWS_PER_CORE, K], f8, kind="ExternalInput").ap()
    al = nc.dram_tensor("al", [ROWS_PER_CORE, K], f8, kind="ExternalInput").ap()
    # B per slot: row (ki*8 + ko) holds the 1024 n-values; hi and lo.
    bhs = [
        nc.dram_tensor(f"bh{s}", [P * KK, N], f8, kind="ExternalInput").ap()
        for s in range(3)
    ]
    bls = [
        nc.dram_tensor(f"bl{s}", [P * KK, N], f8, kind="ExternalInput").ap()
        for s in range(3)
    ]
    d = nc.dram_tensor("d", [ROWS_PER_CORE, N], bf16, kind="ExternalOutput").ap()

    # which slot (-> b input) each m-tile uses (static, uniform across cores)
    tile_slot = []
    for s in range(3):
        tile_slot += [s] * SLOT_TILES[s]

    from contextlib import ExitStack

    with tile.TileContext(nc) as tc, ExitStack() as ctx:
        bpool = ctx.enter_context(tc.tile_pool(name="bpool", bufs=1))
        apool = ctx.enter_context(tc.tile_pool(name="apool", bufs=4))
        psd = ctx.enter_context(tc.tile_pool(name="psd", bufs=8, space="PSUM"))
        # Deep store staging: early DMA-engine time is monopolized by the
        # B/A loads, so d-stores queue up; 24 bufs (48KB) of slack keep the
        # eviction copies (and thus PSUM recycling) from backpressuring PE.
        dpool = ctx.enter_context(tc.tile_pool(name="dpool", bufs=24))

        # First load issued before anything else: b0h[ko0-3] on the sync
        # HWDGE queue, so SP's DMA issue chain starts at t=0.
        HB = KK * N // 2
        bt00 = bpool.tile([P, KK, N], f8, name="b0_0")
        b00f = bt00[:].rearrange("ki ko n -> ki (ko n)")
        b00in = bhs[0].rearrange("(ki ko) n -> ki (ko n)", ko=KK)
        nc.sync.dma_start(out=b00f[:, :HB], in_=b00in[:, :HB])

        # Warmup: the PE p-state ramps to full clock only after 3us of
        # continuous execution. Dummy DoubleRow matmuls on zeroed tiles
        # keep the PE busy (and ramping) while the first B/A DMAs land,
        # so the real matmul stream starts at full speed with no idle gap.
        wa = bpool.tile([P, 2, P], f8, name="wa")
        wb = bpool.tile([P, 2, 256], f8, name="wb")
        nc.vector.memset(wa[:], 0.0)
        nc.vector.memset(wb[:], 0.0)
        wp = psd.tile([P, 512], f32, name="ps")
        for _ in range(WARMUP):
            nc.tensor.matmul(
                wp[:, :256],
                wa[:],
                wb[:],
                start=True,
                stop=True,
                perf_mode=mybir.MatmulPerfMode.DoubleRow,
            )

        b_sb = {}  # (slot, lvl) -> [128, KK, N] fp8 tile

        def load_b(s, lvl, pieces=None, queues=None):
            src = (bhs if lvl == 0 else bls)[s]
            bt = bpool.tile([P, KK, N], f8, name=f"b{lvl}_{s}")
            # Flat [128, 8KB] view: per-partition lines are contiguous in
            # both DRAM and SBUF, so the DMA needs 128 descriptors, not
            # 1024 — shorter SWDGE descriptor-generation on the Pool SEQ.
            # Lo planes: ko-tiles >= 2*BL_PAIRS are never read (the b_l
            # correction only covers BL_PAIRS k-pairs), so don't load them.
            out_f = bt[:].rearrange("ki ko n -> ki (ko n)")
            in_f = src.rearrange("(ki ko) n -> ki (ko n)", ko=KK)
            end = (KK if lvl == 0 else 2 * BL_PAIRS) * N
            pieces = pieces or [(0, end)]
            for i, (p0, p1) in enumerate(pieces):
                queue = (queues or {}).get(i, nc.gpsimd)
                queue.dma_start(out=out_f[:, p0:p1], in_=in_f[:, p0:p1])
            b_sb[(s, lvl)] = bt

        a_ch = {}  # (chunk, lvl) -> [128, CHUNK, KK, 128] fp8 tile

        def load_chunk(c, pieces=None, queues=None, lvls=(0, 1)):
            # pieces: tile sub-ranges loaded as separate DMAs so their
            # consumers (tracked per sub-tile range) unblock early.
            for lvl in lvls:
                pool, src = ((apool, ah), (apool, al))[lvl]
                at = a_ch.get((c, lvl))
                if at is None:
                    at = pool.tile([P, CHUNK, KK, P], f8, name=f"a{lvl}")
                    a_ch[(c, lvl)] = at
                for i, (p0, p1) in enumerate(pieces or [(0, CHUNK)]):
                    queue = (queues or {}).get(i, nc.gpsimd)
                    queue.dma_start(
                        out=at[:, p0:p1],
                        in_=src[
                            (c * CHUNK + p0) * P : (c * CHUNK + p1) * P, :
                        ].rearrange("(c ki) (ko mm) -> ki c ko mm", ki=P, ko=KK),
                    )

        # Startup: the serialized DMA train is ordered so that each piece
        # lands just before its first consumer instructions need it, with
        # the two lead pieces on HWDGE queues (shorter issue lead than a
        # SWDGE prep). Desired grant order: b0h[ko0-3] (sync), a0_hi
        # tiles0-1 (scalar), then the SWDGE train: b0h[ko4-7], a0_lo
        # tiles0-1, b0l in halves, a0 tiles2-4, b1, chunks 1-2, b2 later.
        # Grant order on the serialized DMA engine follows request order:
        # b0h[ko0-3] via sync HWDGE (shortest lead), then the SWDGE preps
        # in emission order. The first A piece is prepped BEFORE b0h's
        # second half so the first matmuls (jj0/jj1 of tiles 0-1) can
        # start ~1.4us earlier; the scheduler hoists them over the wait
        # for b0h[ko4-7].
        b_sb[(0, 0)] = bt00
        load_chunk(0, pieces=[(0, 4)], lvls=(0,))
        nc.gpsimd.dma_start(out=b00f[:, HB:], in_=b00in[:, HB:])
        load_chunk(0, pieces=[(0, 4)], lvls=(1,))
        load_b(0, 1)
        load_chunk(0, pieces=[(4, CHUNK)])
        load_b(1, 0)
        load_chunk(1)
        load_b(1, 1)
        load_chunk(2)

        for t in range(TILES_PER_CORE):
            c, j = divmod(t, CHUNK)
            if j == 0 and c + PREFETCH < NCHUNKS:
                load_chunk(c + PREFETCH)
            if t == 8:
                load_b(2, 0)
                load_b(2, 1)
            s = tile_slot[t]
            at_h = a_ch[(c, 0)]
            at_l = a_ch[(c, 1)]
            b_h = b_sb[(s, 0)]
            b_l = b_sb[(s, 1)]
            last = t == TILES_PER_CORE - 1
            ps0 = psd.tile([P, 512], f32, name="ps")
            if not last:
                ps1 = psd.tile([P, 512], f32, name="ps")
                chains = ((ps0[:], 0, 512), (ps1[:], 512, 1024))
            else:
                # Last tile: nh1 as two 256-wide chains (same PE cycles) so
                # the first half's eviction+store pipeline overlaps the
                # second half's matmuls, shortening the kernel tail. psq0
                # reuses the warmup bank (free after warmup).
                psq0 = psd.tile([P, 512], f32, name="ps")
                psq1 = psd.tile([P, 512], f32, name="ps")
                chains = (
                    (ps0[:], 0, 512),
                    (psq0[:, :256], 512, 768),
                    (psq1[:, :256], 768, 1024),
                )
            blp = 2 if t in BL2_TILES else BL_PAIRS
            nchain = 2 * (KK // 2) + blp
            for pst, n0, n1 in chains:
                idx = 0
                for w_t, r_t, npairs in (
                    (at_h, b_h, KK // 2),
                    (at_l, b_h, KK // 2),
                    (at_h, b_l, blp),
                ):
                    for jj in range(npairs):
                        nc.tensor.matmul(
                            pst,
                            w_t[:, j, 2 * jj : 2 * jj + 2, :],
                            r_t[:, 2 * jj : 2 * jj + 2, n0:n1],
                            start=(idx == 0),
                            stop=(idx == nchain - 1),
                            perf_mode=mybir.MatmulPerfMode.DoubleRow,
                        )
                        idx += 1
            d_sb = dpool.tile([P, N], bf16, name="d_sb")
            if not last:
                nc.vector.tensor_copy(d_sb[:, :512], ps0[:])
                nc.vector.tensor_copy(d_sb[:, 512:], ps1[:])
                nc.sync.dma_start(out=d[t * P : (t + 1) * P, :], in_=d_sb[:])
            else:
                # Each piece is evicted and stored as soon as its chain
                # stops; only the final 256-wide piece trails the last
                # matmul.
                nc.vector.tensor_copy(d_sb[:, :512], ps0[:])
                nc.sync.dma_start(
                    out=d[t * P : (t + 1) * P, :512], in_=d_sb[:, :512]
                )
                nc.vector.tensor_copy(d_sb[:, 512:768], psq0[:, :256])
                nc.vector.tensor_copy(d_sb[:, 768:], psq1[:, :256])
                nc.sync.dma_start(
                    out=d[t * P : (t + 1) * P, 512:], in_=d_sb[:, 512:]
                )
            # free the chunk dict entries we no longer need
            if j == CHUNK - 1:
                a_ch.pop((c, 0), None)
                a_ch.pop((c, 1), None)

    nc.compile()
    return nc


_NC_CACHE = None


def _prep_inputs(a, b):
    """Host-side shard + transpose + fp8 hi/lo split. Returns in_maps."""
    import ml_dtypes

    f8 = ml_dtypes.float8_e4m3

    a32 = np.ascontiguousarray(np.asarray(a), dtype=np.float32)
    b32 = np.ascontiguousarray(np.asarray(b), dtype=np.float32)
    assert a32.shape == (M_TOTAL, K), a32.shape
    assert b32.shape == (E, K, N), b32.shape

    a_h = a32.astype(f8)
    a_l = (a32 - a_h.astype(np.float32)).astype(f8)
    b_h = b32.astype(f8)
    b_l = (b32 - b_h.astype(np.float32)).astype(f8)

    # Per-expert B in [ki, ko, n] lhs-contraction layout, flattened 2D.
    def prep_b(x):  # x: [K, N] fp8
        return np.ascontiguousarray(
            x.reshape(KK, P, N).transpose(1, 0, 2).reshape(P * KK, N)
        )

    b_h_prep = [prep_b(b_h[e]) for e in range(E)]
    b_l_prep = [prep_b(b_l[e]) for e in range(E)]

    # Per-core A shards (zero-padded), then per-tile transpose to
    # [t, ki, ko, mm] flattened to [(t ki), (ko mm)].
    def prep_a(x):  # x: [ROWS_PER_CORE, K] fp8
        y = x.reshape(TILES_PER_CORE, P, KK, P).transpose(0, 3, 2, 1)
        return np.ascontiguousarray(y).reshape(ROWS_PER_CORE, K)

    in_maps = []
    for c in range(8):
        sh_h = np.zeros((ROWS_PER_CORE, K), dtype=f8)
        sh_l = np.zeros((ROWS_PER_CORE, K), dtype=f8)
        for cc, s, soff, goff, n in _SCHEDULE:
            if cc == c:
                sh_h[soff : soff + n] = a_h[goff : goff + n]
                sh_l[soff : soff + n] = a_l[goff : goff + n]
        m = {"ah": prep_a(sh_h), "al": prep_a(sh_l)}
        for s in range(3):
            e = SLOT_EXPERT[s][c]
            m[f"bh{s}"] = b_h_prep[e]
            m[f"bl{s}"] = b_l_prep[e]
        in_maps.append(m)
    return in_maps


def kernel(a, b):
    global _NC_CACHE
    from concourse.bass_utils import run_bass_kernel_spmd

    if _NC_CACHE is None:
        _NC_CACHE = _build_bass()
    nc = _NC_CACHE

    in_maps = _prep_inputs(a, b)
    res = run_bass_kernel_spmd(nc, in_maps, core_ids=list(range(8)))

    out = np.empty((M_TOTAL, N), dtype=np.float32)
    for c, s, soff, goff, n in _SCHEDULE:
        out[goff : goff + n] = res.results[c]["d"][soff : soff + n].astype(
            np.float32
        )
    return out
